# revision 8
# baseline (speedup 1.0000x reference)
"""Trainium2 Bass kernel for a dense transformer block.

Strategy: data-parallel over tokens. The [2, 2048, 1024] input is split
into 8 shards of 512 tokens (4 shards per batch element, one per core).
Each core runs LN1 -> QKV -> attention -> proj(+res) -> LN2 -> MLP(+res)
for its own tokens. Attention needs K/V for the whole 2048-token
sequence of the core's batch element, so after the QKV projection the
cores AllGather K/V within their 4-core batch group; everything else is
fully local. Weights are replicated per core and streamed from HBM.

Layout: activations live in SBUF as [feature(partition), token(free)]
tiles of [128, 512]; weights are pre-transposed and pre-tiled on the
host so every matmul is a plain lhsT[K=128, M=128] x rhs[K=128, N=512].
The residual spine runs in fp32 with float32r matmuls (FP22 multiply,
fp32 accumulate, full PE rate at N >= 256); Q/K/V, the softmax
probabilities, the MLP hidden activations and mlp_w2 are fp16 (also
full PE rate, ~1e-4 relative rounding).  V is produced directly in
[token, feature] layout (x as the stationary operand) so the attention
P@V matmul needs no transposes; a ones-column appended to V yields the
softmax denominators in the same accumulation (M=65).
"""

import contextlib

import numpy as np

import concourse.bass as bass  # noqa: F401
import concourse.mybir as mybir
import concourse.tile as tile
from concourse import bacc
from concourse import bass_utils

F32 = mybir.dt.float32
F32R = mybir.dt.float32r
F16 = mybir.dt.float16
AF = mybir.ActivationFunctionType

DIM = 1024
HEADS = 16
HD = 64
HIDDEN = 4096
B = 2
L = 2048
N_CORES = 8
TOK = 512           # tokens per core
DT = DIM // 128     # 8 feature tiles
HT = HIDDEN // 128  # 32 hidden tiles
NPAIR = HEADS // 2  # 8 head pairs (128 features each)
RANKS = 4           # cores per batch group


def _r(ap):
    return ap.bitcast(F32R)


def _emit_ln(nc, ones, x_tiles, out_pool, tmp_pool, small_pool, bc_pool, ps_pool):
    """LayerNorm over the partition (feature) axis of 8 [128, 512] tiles.

    Stats come from two ones-matmul accumulation chains (sum and sum of
    squares reduce over partitions on the PE); the normalization is two
    DVE passes against partition-broadcast scale/shift rows.
    Returns 8 normalized tiles allocated from out_pool (tag "norm").
    """
    sum_ps = ps_pool.tile([1, TOK], F32, tag="lnps")
    sq_ps = ps_pool.tile([1, TOK], F32, tag="lnps")
    sq_tiles = []
    for dc in range(DT):
        sq = tmp_pool.tile([128, TOK], F32, tag="lntmp")
        nc.vector.tensor_mul(_r(sq[:]), x_tiles[dc][:], x_tiles[dc][:])
        sq_tiles.append(sq)
    for dc in range(DT):
        nc.tensor.matmul(sum_ps[:], _r(ones[:]), _r(x_tiles[dc][:]),
                         start=(dc == 0), stop=(dc == DT - 1))
    for dc in range(DT):
        nc.tensor.matmul(sq_ps[:], _r(ones[:]), _r(sq_tiles[dc][:]),
                         start=(dc == 0), stop=(dc == DT - 1))

    mean = small_pool.tile([1, TOK], F32, tag="lnsc")
    ex2 = small_pool.tile([1, TOK], F32, tag="lnsc")
    var = small_pool.tile([1, TOK], F32, tag="lnsc")
    std = small_pool.tile([1, TOK], F32, tag="lnsc")
    rstd = small_pool.tile([1, TOK], F32, tag="lnsc")
    bsh = small_pool.tile([1, TOK], F32, tag="lnsc")
    nc.vector.tensor_scalar_mul(mean[:], sum_ps[:], 1.0 / DIM)
    nc.vector.tensor_scalar_mul(ex2[:], sq_ps[:], 1.0 / DIM)
    nc.vector.tensor_mul(var[:], mean[:], mean[:])
    nc.vector.tensor_sub(var[:], ex2[:], var[:])
    nc.scalar.activation(std[:], var[:], AF.Sqrt)
    nc.vector.reciprocal(rstd[:], std[:])
    nc.vector.tensor_mul(bsh[:], mean[:], rstd[:])

    a_bc = bc_pool.tile([128, TOK], F32, tag="lnbc")
    b_bc = bc_pool.tile([128, TOK], F32, tag="lnbc")
    nc.gpsimd.partition_broadcast(a_bc[:], rstd[:])
    nc.gpsimd.partition_broadcast(b_bc[:], bsh[:])

    out_tiles = []
    for dc in range(DT):
        y = out_pool.tile([128, TOK], F32, tag="norm")
        nc.vector.tensor_mul(_r(y[:]), x_tiles[dc][:], a_bc[:])
        nc.vector.tensor_sub(_r(y[:]), y[:], b_bc[:])
        out_tiles.append(y)
    return out_tiles


def build():
    nc = bacc.Bacc("TRN2", target_bir_lowering=False, debug=False,
                   num_devices=N_CORES)

    xT = nc.dram_tensor("xT", [DIM, TOK], F32, kind="ExternalInput").ap()
    # lhsT-tiled weights: [m_tiles, 128(k_inner), k_tiles, 128(m_inner)]
    wqk = nc.dram_tensor("wqk", [16, 128, DT, 128], F32, kind="ExternalInput").ap()
    wv = nc.dram_tensor("wv", [DT, 128, DIM], F32, kind="ExternalInput").ap()
    wproj = nc.dram_tensor("wproj", [DT, 128, DT, 128], F32, kind="ExternalInput").ap()
    w1 = nc.dram_tensor("w1", [HT, 128, DT, 128], F32, kind="ExternalInput").ap()
    w2 = nc.dram_tensor("w2", [DT, 128, HT, 128], F16, kind="ExternalInput").ap()
    yT = nc.dram_tensor("yT", [DIM, TOK], F32, kind="ExternalOutput").ap()

    with tile.TileContext(nc) as tc:
        with contextlib.ExitStack() as ctx:
            # ---- long-lived pools -------------------------------------
            const = ctx.enter_context(tc.tile_pool(name="const", bufs=1))
            norm = ctx.enter_context(tc.tile_pool(name="norm", bufs=8))
            ax = ctx.enter_context(tc.tile_pool(name="ax", bufs=16))
            small = ctx.enter_context(tc.tile_pool(name="small", bufs=8))
            bc = ctx.enter_context(tc.tile_pool(name="bc", bufs=2))
            bcr_p = ctx.enter_context(tc.tile_pool(name="bcr", bufs=4))
            tmp = ctx.enter_context(tc.tile_pool(name="tmp", bufs=2))
            dram = ctx.enter_context(tc.tile_pool(name="dram", bufs=1, space="DRAM"))

            ones = const.tile([128, 1], F32)
            nc.vector.memset(ones[:], 1.0)

            kv_in = dram.tile([16, 128, TOK], F16)
            kv_out = dram.tile([RANKS, 16, 128, TOK], F16)

            with contextlib.ExitStack() as octx:
                xp = octx.enter_context(tc.tile_pool(name="xp", bufs=8))
                qp_pool = octx.enter_context(tc.tile_pool(name="qp", bufs=8))

                # ---- load x shard -------------------------------------
                x_tiles = []
                for dc in range(DT):
                    t = xp.tile([128, TOK], F32, tag="x")
                    nc.sync.dma_start(out=_r(t[:]), in_=_r(xT[dc * 128:(dc + 1) * 128, :]))
                    x_tiles.append(t)

                # ---- phase 1: LN1, QKV projection, KV allgather -------
                with tc.tile_pool(name="wqk", bufs=4) as wqk_pool, \
                     tc.tile_pool(name="wv", bufs=8) as wv_pool, \
                     tc.tile_pool(name="kvtmp", bufs=4) as kvtmp, \
                     tc.tile_pool(name="ps1", bufs=4, space="PSUM") as ps1, \
                     tc.tile_pool(name="lnps", bufs=2, space="PSUM") as lnps:

                    ln1x = _emit_ln(nc, ones, x_tiles, norm, tmp, small, bc, lnps)

                    # K etiles first (8..15), then V, so the allgather
                    # can run while Q (etiles 0..7) is computed.
                    def qk_etile(et, dest_tile):
                        wt = wqk_pool.tile([128, DT, 128], F32, tag="wqk")
                        nc.sync.dma_start(out=_r(wt[:]), in_=_r(wqk[et]))
                        ps = ps1.tile([128, TOK], F32, tag="mm")
                        for dc in range(DT):
                            nc.tensor.matmul(ps[:], _r(wt[:, dc, :]),
                                             _r(ln1x[dc][:]),
                                             start=(dc == 0), stop=(dc == DT - 1))
                        nc.vector.tensor_copy(dest_tile[:], ps[:])

                    for et in range(8, 16):  # K rows
                        kt_sb = kvtmp.tile([128, TOK], F16, tag="kv")
                        qk_etile(et, kt_sb)
                        nc.sync.dma_start(out=kv_in[et - 8], in_=kt_sb[:])

                    # V in [token, feature] layout: x as the stationary
                    # operand, W_v^T as the moving one.
                    for nh in range(2):
                        wv_tiles = []
                        for dc in range(DT):
                            wvt = wv_pool.tile([128, TOK], F32, tag="wv")
                            nc.sync.dma_start(
                                out=_r(wvt[:]),
                                in_=_r(wv[dc, :, nh * 512:(nh + 1) * 512]))
                            wv_tiles.append(wvt)
                        for tt in range(TOK // 128):
                            ps = ps1.tile([128, TOK], F32, tag="mm")
                            for dc in range(DT):
                                nc.tensor.matmul(
                                    ps[:],
                                    _r(ln1x[dc][:, tt * 128:(tt + 1) * 128]),
                                    _r(wv_tiles[dc][:]),
                                    start=(dc == 0), stop=(dc == DT - 1))
                            vt_sb = kvtmp.tile([128, TOK], F16, tag="kv")
                            nc.vector.tensor_copy(vt_sb[:], ps[:])
                            nc.sync.dma_start(out=kv_in[8 + tt * 2 + nh],
                                              in_=vt_sb[:])

                    nc.gpsimd.collective_compute(
                        "AllGather",
                        mybir.AluOpType.bypass,
                        replica_groups=[[0, 1, 2, 3], [4, 5, 6, 7]],
                        ins=[kv_in.opt()],
                        outs=[kv_out.opt()],
                    )

                    q_tiles = []
                    for et in range(8):  # Q rows
                        qt = qp_pool.tile([128, TOK], F16, tag="q")
                        qk_etile(et, qt)
                        q_tiles.append(qt)

                # ---- phase 2: attention -------------------------------
                attn_tiles = []
                with tc.tile_pool(name="kp", bufs=3) as kp_pool, \
                     tc.tile_pool(name="vaug", bufs=4) as vaug_pool, \
                     tc.tile_pool(name="exps", bufs=3) as exp_pool, \
                     tc.tile_pool(name="pss", bufs=2, space="PSUM") as pss, \
                     tc.tile_pool(name="pso", bufs=4, space="PSUM") as pso:
                    for p in range(NPAIR):
                        qp = q_tiles[p]
                        o0 = pso.tile([HD + 1, TOK], F32, tag="pso")
                        o1 = pso.tile([HD + 1, TOK], F32, tag="pso")
                        nh, coff = p // 4, (p % 4) * 128
                        for r_i in range(RANKS):
                            kp = kp_pool.tile([128, TOK], F16, tag="kp")
                            nc.sync.dma_start(out=kp[:], in_=kv_out[r_i, p])
                            for tt in range(4):
                                kt = r_i * 4 + tt
                                va = vaug_pool.tile([128, 2, HD + 1], F16,
                                                    tag="va")
                                nc.sync.dma_start(
                                    out=va[:, :, 0:HD],
                                    in_=kv_out[r_i, 8 + tt * 2 + nh, :,
                                               coff:coff + 128].rearrange(
                                                   "t (h d) -> t h d", d=HD))
                                nc.vector.memset(va[:, :, HD:HD + 1], 1.0)

                                ss = pss.tile([128, 2 * TOK], F32, tag="pss")
                                nc.tensor.matmul(
                                    ss[:, 0:TOK],
                                    kp[0:HD, tt * 128:(tt + 1) * 128],
                                    qp[0:HD, :], start=True, stop=True)
                                nc.tensor.matmul(
                                    ss[:, TOK:2 * TOK],
                                    kp[HD:128, tt * 128:(tt + 1) * 128],
                                    qp[HD:128, :], start=True, stop=True)
                                ex = exp_pool.tile([128, 2 * TOK], F16, tag="ex")
                                nc.scalar.activation(ex[:], ss[:], AF.Exp,
                                                     scale=float(HD) ** -0.5)
                                nc.tensor.matmul(o0[:], va[:, 0, :],
                                                 ex[:, 0:TOK],
                                                 start=(kt == 0), stop=(kt == 15))
                                nc.tensor.matmul(o1[:], va[:, 1, :],
                                                 ex[:, TOK:2 * TOK],
                                                 start=(kt == 0), stop=(kt == 15))
                        at = ax.tile([128, TOK], F32, tag="ax")
                        for h_i, o in ((0, o0), (1, o1)):
                            rc = small.tile([1, TOK], F32, tag="rc")
                            nc.vector.reciprocal(rc[:], o[HD:HD + 1, :])
                            bcr = bcr_p.tile([HD, TOK], F32, tag="bcr")
                            nc.gpsimd.partition_broadcast(bcr[:], rc[:])
                            nc.vector.tensor_mul(_r(at[h_i * HD:(h_i + 1) * HD, :]),
                                                 o[0:HD, :], bcr[:])
                        attn_tiles.append(at)

                # ---- phase 3: proj + residual, LN2 --------------------
                X_tiles = []
                with tc.tile_pool(name="wproj", bufs=2) as wp_pool, \
                     tc.tile_pool(name="ps3", bufs=4, space="PSUM") as ps3, \
                     tc.tile_pool(name="lnps2", bufs=2, space="PSUM") as lnps2:
                    for et in range(DT):
                        wt = wp_pool.tile([128, DT, 128], F32, tag="wp")
                        nc.sync.dma_start(out=_r(wt[:]), in_=_r(wproj[et]))
                        ps = ps3.tile([128, TOK], F32, tag="mm")
                        for dc in range(DT):
                            nc.tensor.matmul(ps[:], _r(wt[:, dc, :]),
                                             _r(attn_tiles[dc][:]),
                                             start=(dc == 0), stop=(dc == DT - 1))
                        xt = ax.tile([128, TOK], F32, tag="ax")
                        nc.vector.tensor_add(_r(xt[:]), ps[:], x_tiles[et][:])
                        X_tiles.append(xt)

                    Y_tiles = _emit_ln(nc, ones, X_tiles, norm, tmp, small,
                                       bc, lnps2)
            # xp + qp released here; their space is recycled for h.

            # ---- phase 4: MLP + residual ------------------------------
            with tc.tile_pool(name="hp", bufs=32) as hp, \
                 tc.tile_pool(name="w1", bufs=4) as w1_pool, \
                 tc.tile_pool(name="w2", bufs=2) as w2_pool, \
                 tc.tile_pool(name="ps4", bufs=4, space="PSUM") as ps4:
                h_tiles = []
                for ht in range(HT):
                    wt = w1_pool.tile([128, DT, 128], F32, tag="w1")
                    nc.sync.dma_start(out=_r(wt[:]), in_=_r(w1[ht]))
                    ps = ps4.tile([128, TOK], F32, tag="mm")
                    for dc in range(DT):
                        nc.tensor.matmul(ps[:], _r(wt[:, dc, :]),
                                         _r(Y_tiles[dc][:]),
                                         start=(dc == 0), stop=(dc == DT - 1))
                    h = hp.tile([128, TOK], F16, tag="h")
                    nc.scalar.activation(h[:], ps[:], AF.Gelu)
                    h_tiles.append(h)

                for et in range(DT):
                    wt = w2_pool.tile([128, HT, 128], F16, tag="w2")
                    nc.sync.dma_start(out=wt[:], in_=w2[et])
                    ps = ps4.tile([128, TOK], F32, tag="mm")
                    for hc in range(HT):
                        nc.tensor.matmul(ps[:], wt[:, hc, :], h_tiles[hc][:],
                                         start=(hc == 0), stop=(hc == HT - 1))
                    ot = norm.tile([128, TOK], F32, tag="norm")
                    nc.vector.tensor_add(ot[:], ps[:], X_tiles[et][:])
                    nc.sync.dma_start(out=yT[et * 128:(et + 1) * 128, :],
                                      in_=ot[:])

    nc.compile()
    return nc


def _tile_lhsT(wT, kt, mt, dtype=np.float32):
    """[Ktot, Mtot] -> [mt, 128, kt, 128] so each m-tile is one
    contiguous DMA and [:, :, kc, :] is a [128, 128] lhsT block."""
    return np.ascontiguousarray(
        wT.reshape(kt, 128, mt, 128).transpose(2, 1, 0, 3).astype(dtype))


_CACHE = {}


def kernel(x, ln1_w, ln2_w, qkv_w, proj_w, mlp_w1, mlp_w2):
    x = np.asarray(x, dtype=np.float32)
    ln1_w = np.asarray(ln1_w, dtype=np.float32)
    ln2_w = np.asarray(ln2_w, dtype=np.float32)
    qkv_w = np.asarray(qkv_w, dtype=np.float32)
    proj_w = np.asarray(proj_w, dtype=np.float32)
    mlp_w1 = np.asarray(mlp_w1, dtype=np.float32)
    mlp_w2 = np.asarray(mlp_w2, dtype=np.float32)

    if "nc" not in _CACHE:
        _CACHE["nc"] = build()
    nc = _CACHE["nc"]

    # Fold the LN scales into the consuming weight matrices.
    wqkv = qkv_w * ln1_w[None, :]
    wqk_h = _tile_lhsT(np.ascontiguousarray(wqkv[:2 * DIM].T), DT, 16)
    wv_h = np.ascontiguousarray(wqkv[2 * DIM:].T).reshape(DT, 128, DIM)
    wproj_h = _tile_lhsT(np.ascontiguousarray(proj_w.T), DT, DT)
    w1_h = _tile_lhsT(np.ascontiguousarray((mlp_w1 * ln2_w[None, :]).T), DT, HT)
    w2_h = _tile_lhsT(np.ascontiguousarray(mlp_w2.T), HT, DT, dtype=np.float16)

    xs = x.reshape(B, RANKS, TOK, DIM)
    in_maps = []
    for c in range(N_CORES):
        b, j = divmod(c, RANKS)
        in_maps.append({
            "xT": np.ascontiguousarray(xs[b, j].T),
            "wqk": wqk_h, "wv": wv_h, "wproj": wproj_h,
            "w1": w1_h, "w2": w2_h,
        })

    res = bass_utils.run_bass_kernel_spmd(nc, in_maps,
                                          core_ids=list(range(N_CORES)))
    _CACHE["last_results"] = res

    out = np.empty((B, L, DIM), dtype=np.float32)
    for c in range(N_CORES):
        b, j = divmod(c, RANKS)
        out[b, j * TOK:(j + 1) * TOK, :] = res.results[c]["yT"].T
    return out
